# revision 17
# baseline (speedup 1.0000x reference)
"""Trainium2 Bass kernel for nn_BasicAE (GNN message passing + pairwise decode).

8-core SPMD. Nodes sharded (512/core); edges assigned to the destination-row
core, host-sorted by (row_block, col_block) into a cross-core-uniform padded
template so one program serves all cores. All gathers / segment_sum run on the
TensorEngine via host-built block one-hot matmuls:
  e_hid(ch-major) = A_blk @ RowOneHot + B_cblk @ ColOneHot + w_attr @ attr_row
  m(edge-major)   = relu1_chunk.T @ ew2            (layout flip via lhsT)
  agg.T(ch-major) = m_chunk.T @ AggOneHot          (PSUM-accumulated per block)
B = h@W1b node-major tables are AllGathered (bf16) once per layer.
Decode: d2 = P @ Q.T with K=10 augmentation (P=[-2x,sq,1], Q=[x,1,sq]),
sigmoid fused on ScalarE; diagonal zeroed on host after readback.
"""
import numpy as np
import ml_dtypes

N_NODES = 4096
HID = 128
EMB = 8
NCORES = 8
NPC = N_NODES // NCORES          # 512 nodes per core
RB = NPC // 128                  # 4 row blocks per core
CB = N_NODES // 128              # 32 col blocks (global)
W_DEC = 10.0
B_DEC = -1.0

bf16 = ml_dtypes.bfloat16
_CACHE = {}


def _host_prep(nodes, edges, edge_attr):
    row = np.asarray(edges[0], dtype=np.int64)
    col = np.asarray(edges[1], dtype=np.int64)
    attr = np.asarray(edge_attr, dtype=np.float32).reshape(-1)
    nodes = np.asarray(nodes, dtype=np.float32).reshape(-1)

    core_of = row // NPC
    per_core = []
    counts = np.zeros((NCORES, RB, CB), dtype=np.int64)
    for c in range(NCORES):
        m = core_of == c
        r, cl, at = row[m] - c * NPC, col[m], attr[m]
        rb, cb = r // 128, cl // 128
        order = np.lexsort((cb, rb))
        r, cl, at, rb, cb = r[order], cl[order], at[order], rb[order], cb[order]
        np.add.at(counts[c], (rb, cb), 1)
        per_core.append((r, cl, at, rb, cb))

    G = counts.max(axis=0)
    G = ((G + 3) // 4) * 4                            # [RB, CB]
    blk_sz = ((G.sum(axis=1) + 127) // 128) * 128     # [RB]
    blk_start = np.concatenate([[0], np.cumsum(blk_sz)]).astype(np.int64)
    E_pad = int(blk_start[-1])
    grp_start = np.zeros((RB, CB), dtype=np.int64)
    for b in range(RB):
        grp_start[b] = blk_start[b] + np.concatenate([[0], np.cumsum(G[b])[:-1]])

    # chunk template: per row block, 512-chunks plus one remainder chunk
    chunks = []          # (start, size, rb)
    for b in range(RB):
        s, e = int(blk_start[b]), int(blk_start[b + 1])
        while s < e:
            sz = min(512, e - s)
            chunks.append((s, sz, b))
            s += sz
    col_tmpl = [[] for _ in chunks]
    for ci, (cs, csz, crb) in enumerate(chunks):
        for k in range(CB):
            s, e = int(grp_start[crb, k]), int(grp_start[crb, k] + G[crb, k])
            lo, hi = max(s, cs), min(e, cs + csz)
            if hi > lo:
                col_tmpl[ci].append((k, lo - cs, hi - cs))

    data = []
    for c in range(NCORES):
        r, cl, at, rb, cb = per_core[c]
        fill = grp_start.copy()
        pos = np.zeros(len(r), dtype=np.int64)
        for i in range(len(r)):
            pos[i] = fill[rb[i], cb[i]]
            fill[rb[i], cb[i]] += 1
        row_l = np.zeros(E_pad, dtype=np.int64)
        col_g = np.zeros(E_pad, dtype=np.int64)
        attr_v = np.zeros(E_pad, dtype=np.float32)
        row_full = np.zeros(E_pad, dtype=np.int64)
        real = np.zeros(E_pad, dtype=bool)
        row_l[pos] = r % 128
        col_g[pos] = cl
        attr_v[pos] = at
        row_full[pos] = r + c * NPC
        real[pos] = True
        idx = np.nonzero(real)[0]

        row_oh = np.zeros((128, E_pad), dtype=bf16)
        col_oh = np.zeros((128, E_pad), dtype=bf16)
        agg_oh = np.zeros((128, E_pad), dtype=bf16)
        row_oh[row_l[idx], idx] = 1
        col_oh[col_g[idx] % 128, idx] = 1
        agg_oh[idx % 128, (idx // 128) * 128 + row_l[idx]] = 1

        e0 = np.zeros((4, E_pad), dtype=bf16)
        e0[0] = attr_v.astype(bf16)
        e0[1, idx] = nodes[row_full[idx]].astype(bf16)
        e0[2, idx] = nodes[col_g[idx]].astype(bf16)
        data.append(dict(row_oh=row_oh, col_oh=col_oh, agg_oh=agg_oh, e0=e0,
                         nodesT=nodes[c * NPC:(c + 1) * NPC][None, :].astype(bf16)))
    tmpl = dict(E_pad=E_pad, chunks=chunks, col_tmpl=col_tmpl)
    return tmpl, data


def _prep_weights(gcl_params, fc_w, fc_b):
    ws, zflags = {}, {}
    for l, p in enumerate(gcl_params):
        g = {k: np.asarray(v, dtype=np.float32) for k, v in p.items()}
        if l == 0:
            e1 = np.zeros((4, HID), np.float32)
            e1[0] = g["ew1"][2]
            e1[1] = g["ew1"][0]
            e1[2] = g["ew1"][1]
            ws[f"ew1_{l}"] = e1.astype(bf16)
            ws[f"nw1h_{l}"] = g["nw1"][0:1].astype(bf16)
            ws[f"nw1a_{l}"] = g["nw1"][1:129].astype(bf16)
        else:
            ws[f"w1a_{l}"] = g["ew1"][0:HID].astype(bf16)
            ws[f"w1b_{l}"] = g["ew1"][HID:2 * HID].astype(bf16)
            ws[f"wat_{l}"] = g["ew1"][2 * HID:2 * HID + 1].astype(bf16)
            ws[f"nw1h_{l}"] = g["nw1"][0:HID].astype(bf16)
            ws[f"nw1a_{l}"] = g["nw1"][HID:2 * HID].astype(bf16)
        ws[f"ew2_{l}"] = g["ew2"].astype(bf16)
        ws[f"nw2_{l}"] = g["nw2"].astype(bf16)
        ws[f"eb1_{l}"] = np.ascontiguousarray(g["eb1"][:, None])
        ws[f"nb1_{l}"] = np.ascontiguousarray(g["nb1"][:, None])
        ws[f"nb2_{l}"] = np.ascontiguousarray(g["nb2"][:, None])
        ws[f"eb2row_{l}"] = np.ascontiguousarray(g["eb2"][None, :]).astype(bf16)
        zflags[f"eb2_{l}"] = bool(np.all(g["eb2"] == 0))
        zflags[f"nb2_{l}"] = bool(np.all(g["nb2"] == 0))
    ws["fc_w"] = np.asarray(fc_w, np.float32).astype(bf16)
    ws["fc_b"] = np.ascontiguousarray(np.asarray(fc_b, np.float32)[:, None])
    zflags["fc_b"] = bool(np.all(ws["fc_b"] == 0))
    return ws, zflags


def _wspec():
    import concourse.mybir as mybir
    f32, b16 = mybir.dt.float32, mybir.dt.bfloat16
    sp = []
    for l in range(4):
        if l == 0:
            sp += [(f"ew1_{l}", [4, HID], b16), (f"nw1h_{l}", [1, HID], b16)]
        else:
            sp += [(f"w1a_{l}", [HID, HID], b16), (f"w1b_{l}", [HID, HID], b16),
                   (f"wat_{l}", [1, HID], b16), (f"nw1h_{l}", [HID, HID], b16)]
        sp += [(f"nw1a_{l}", [HID, HID], b16), (f"ew2_{l}", [HID, HID], b16),
               (f"nw2_{l}", [HID, HID], b16), (f"eb1_{l}", [HID, 1], f32),
               (f"nb1_{l}", [HID, 1], f32), (f"nb2_{l}", [HID, 1], f32),
               (f"eb2row_{l}", [1, HID], b16)]
    sp += [("fc_w", [HID, EMB], b16), ("fc_b", [EMB, 1], f32)]
    return sp


def _build(tmpl, zflags):
    import concourse.mybir as mybir
    import concourse.tile as tile
    from concourse import bacc
    from concourse.masks import make_identity

    E_pad = tmpl["E_pad"]
    chunks = tmpl["chunks"]
    col_tmpl = tmpl["col_tmpl"]
    f32, b16 = mybir.dt.float32, mybir.dt.bfloat16
    Relu = mybir.ActivationFunctionType.Relu
    Sigm = mybir.ActivationFunctionType.Sigmoid
    Copy = mybir.ActivationFunctionType.Copy

    nc = bacc.Bacc(None, target_bir_lowering=False, debug=False)

    d_rowoh = nc.dram_tensor("row_oh", [128, E_pad], b16, kind="ExternalInput")
    d_coloh = nc.dram_tensor("col_oh", [128, E_pad], b16, kind="ExternalInput")
    d_aggoh = nc.dram_tensor("agg_oh", [128, E_pad], b16, kind="ExternalInput")
    d_e0 = nc.dram_tensor("e0", [4, E_pad], b16, kind="ExternalInput")
    d_nodesT = nc.dram_tensor("nodesT", [1, NPC], b16, kind="ExternalInput")
    wspec = _wspec()
    d_w = {nm: nc.dram_tensor(nm, shp, dt, kind="ExternalInput")
           for nm, shp, dt in wspec}
    d_adj = nc.dram_tensor("adj_rows", [NPC, N_NODES], f32, kind="ExternalOutput")
    d_qt = nc.dram_tensor("qt_shard", [10, NPC], f32, kind="ExternalOutput")

    d_warm_in = nc.dram_tensor("warm_in", [1, 64], f32)
    d_warm_out = nc.dram_tensor("warm_out", [NCORES, 64], f32, addr_space="Shared")
    d_bsh = {l: nc.dram_tensor(f"bsh_{l}", [NPC, HID], b16) for l in (1, 2, 3)}
    d_btab = {l: nc.dram_tensor(f"btab_{l}", [NCORES, NPC, HID], b16,
                                addr_space="Shared") for l in (1, 2, 3)}
    d_qin = nc.dram_tensor("q_in", [10, NPC], f32)
    d_pt = nc.dram_tensor("p_t", [10, NPC], f32)
    d_qout = nc.dram_tensor("q_out", [NCORES, 10, NPC], f32, addr_space="Shared")
    RG = [list(range(NCORES))]

    with tile.TileContext(nc) as tc:
        with (
            tc.tile_pool(name="big", bufs=1) as bigp,
            tc.tile_pool(name="wp", bufs=1) as wp,
            tc.tile_pool(name="work", bufs=2) as work,
            tc.tile_pool(name="psA", bufs=2, space="PSUM") as psA,
            tc.tile_pool(name="psB", bufs=2, space="PSUM") as psB,
            tc.tile_pool(name="psT", bufs=1, space="PSUM") as psT,
            tc.tile_pool(name="psG", bufs=1, space="PSUM") as psG,
        ):
            zsm = wp.tile([1, 64], f32)
            nc.gpsimd.memset(zsm[:], 0.0)
            nc.sync.dma_start(d_warm_in[:], zsm[:])
            nc.gpsimd.collective_compute(
                "AllGather", mybir.AluOpType.bypass, replica_groups=RG,
                ins=[d_warm_in[:]], outs=[d_warm_out[:]])
            row_oh = bigp.tile([128, E_pad], b16)
            col_oh = bigp.tile([128, E_pad], b16)
            e0 = bigp.tile([4, E_pad], b16)
            tableB = bigp.tile([128, CB, 128], b16)
            nc.sync.dma_start(e0[:], d_e0[:])
            nodesT = wp.tile([1, NPC], b16)
            nc.sync.dma_start(nodesT[:], d_nodesT[:])
            w = {}
            for nm, shp, dt in wspec:
                w[nm] = wp.tile(shp, dt, tag=nm, name=nm)
                nc.sync.dma_start(w[nm][:], d_w[nm][:])
            ident = wp.tile([128, 128], b16)
            make_identity(nc, ident)
            ones1 = wp.tile([1, 128], b16)
            nc.gpsimd.memset(ones1[:], 1.0)
            hT = nodesT
            for l in range(4):
                if l == 1:
                    nc.scalar.dma_start(row_oh[:], d_rowoh[:])
                    nc.scalar.dma_start(col_oh[:], d_coloh[:])
                if l > 0:
                    psa = psA.tile([128, 512], f32, tag="p512")
                    nc.tensor.matmul(psa[:], w[f"w1a_{l}"][:], hT[:],
                                     start=True, stop=True)
                    aT = work.tile([128, NPC], b16, tag="aT")
                    nc.scalar.activation(aT[:], psa[:], Copy)
                    tableA = work.tile([128, RB, 128], b16, tag="tableA")
                    for k in range(RB):
                        pst = psT.tile([128, 128], b16, tag="tp")
                        nc.tensor.matmul(pst[:], aT[:, k * 128:(k + 1) * 128],
                                         ident[:], is_transpose=True,
                                         start=True, stop=True)
                        nc.vector.tensor_copy(tableA[:, k, :], pst[:])

                agg_ps = psG.tile([128, RB * 128], f32, tag="agg")
                aggoh_blk = None
                blk0 = 0
                for ci, (cs, csz, crb) in enumerate(chunks):
                    sl = slice(cs, cs + csz)
                    if ci == 0 or chunks[ci - 1][2] != crb:
                        blk0 = cs
                        bsz = sum(z for (_, z, b) in chunks if b == crb)
                        mxb = max(sum(z for (_, z, b) in chunks if b == bb)
                                  for bb in range(RB))
                        aggoh_blk = work.tile([128, mxb], b16, tag="aggohB",
                                              bufs=2)
                        nc.sync.dma_start(aggoh_blk[:, :bsz],
                                          d_aggoh[:, blk0:blk0 + bsz])
                    ps = psA.tile([128, 512], f32, tag="p512")
                    if l == 0:
                        nc.tensor.matmul(ps[:, :csz], w["ew1_0"][:], e0[:, sl],
                                         start=True, stop=True)
                    else:
                        nc.tensor.matmul(ps[:, :csz], tableA[:, crb, :],
                                         row_oh[:, sl], start=True, stop=False)
                        nc.tensor.matmul(ps[:, :csz], w[f"wat_{l}"][:],
                                         e0[0:1, sl], start=False, stop=False)
                        segs = col_tmpl[ci]
                        for i, (cb, lo, hi) in enumerate(segs):
                            nc.tensor.matmul(
                                ps[:, lo:hi], tableB[:, cb, :],
                                col_oh[:, cs + lo:cs + hi],
                                start=False, stop=(i == len(segs) - 1))
                    relu1 = work.tile([128, 512], b16, tag="relu1", bufs=3)
                    nc.scalar.activation(relu1[:, :csz], ps[:, :csz], Relu,
                                         bias=w[f"eb1_{l}"][:])
                    aggoh_t = aggoh_blk[:, cs - blk0:cs - blk0 + csz]
                    ps2 = psB.tile([128, 512], f32, tag="m")
                    nsub = csz // 128
                    for k in range(nsub):
                        ksl = slice(k * 128, (k + 1) * 128)
                        nc.tensor.matmul(ps2[:, ksl], relu1[:, ksl],
                                         w[f"ew2_{l}"][:], start=True,
                                         stop=zflags[f"eb2_{l}"])
                        if not zflags[f"eb2_{l}"]:
                            nc.tensor.matmul(ps2[:, ksl], ones1[:],
                                             w[f"eb2row_{l}"][:],
                                             start=False, stop=True)
                    mt = work.tile([128, 512], b16, tag="m_sb", bufs=3)
                    nc.scalar.activation(mt[:, :csz], ps2[:, :csz], Relu)
                    first = ci == 0 or chunks[ci - 1][2] != crb
                    last = ci == len(chunks) - 1 or chunks[ci + 1][2] != crb
                    for k in range(nsub):
                        ksl = slice(k * 128, (k + 1) * 128)
                        nc.tensor.matmul(agg_ps[:, crb * 128:(crb + 1) * 128],
                                         mt[:, ksl], aggoh_t[:, ksl],
                                         start=(first and k == 0),
                                         stop=(last and k == nsub - 1))
                aggT = work.tile([128, NPC], b16, tag="aggT")
                nc.scalar.activation(aggT[:], agg_ps[:], Copy)

                psh = psA.tile([128, 512], f32, tag="p512")
                nc.tensor.matmul(psh[:], w[f"nw1h_{l}"][:], hT[:],
                                 start=True, stop=False)
                nc.tensor.matmul(psh[:], w[f"nw1a_{l}"][:], aggT[:],
                                 start=False, stop=True)
                hid = work.tile([128, NPC], b16, tag="nhid")
                nc.scalar.activation(hid[:], psh[:], Relu, bias=w[f"nb1_{l}"][:])
                psh2 = psB.tile([128, 512], f32, tag="m")
                nc.tensor.matmul(psh2[:], w[f"nw2_{l}"][:], hid[:],
                                 start=True, stop=True)
                hT_new = work.tile([128, NPC], b16, tag=f"hT_{l % 2}")
                nc.scalar.activation(hT_new[:], psh2[:], Copy)
                if not zflags[f"nb2_{l}"]:
                    nc.vector.tensor_scalar_add(hT_new[:], hT_new[:],
                                                w[f"nb2_{l}"][:])
                hT = hT_new

                if l < 3:
                    lb = l + 1
                    psb = psA.tile([128, 512], f32, tag="p512")
                    nc.tensor.matmul(psb[:], w[f"w1b_{lb}"][:], hT[:],
                                     start=True, stop=True)
                    bT = work.tile([128, NPC], b16, tag="bT")
                    nc.scalar.activation(bT[:], psb[:], Copy)
                    bnm = work.tile([128, RB, 128], b16, tag="bnm")
                    for k in range(RB):
                        pst = psT.tile([128, 128], b16, tag="tp")
                        nc.tensor.matmul(pst[:], bT[:, k * 128:(k + 1) * 128],
                                         ident[:], is_transpose=True,
                                         start=True, stop=True)
                        nc.vector.tensor_copy(bnm[:, k, :], pst[:])
                    nc.sync.dma_start(
                        d_bsh[lb].rearrange("(k p) c -> p k c", p=128), bnm[:])
                    nc.gpsimd.collective_compute(
                        "AllGather", mybir.AluOpType.bypass, replica_groups=RG,
                        ins=[d_bsh[lb][:]], outs=[d_btab[lb][:]])
                    nc.sync.dma_start(
                        tableB[:].rearrange("p (r k) c -> p r k c", r=NCORES),
                        d_btab[lb].rearrange("r (k p) c -> p r k c", p=128))

            # ---- decode ----
            psx = psA.tile([128, 512], f32, tag="p512")
            nc.tensor.matmul(psx[:EMB, :], w["fc_w"][:], hT[:],
                             start=True, stop=True)
            xT = work.tile([EMB, NPC], f32, tag="xT", bufs=1)
            nc.scalar.activation(xT[:], psx[:EMB, :], Copy)
            if not zflags["fc_b"]:
                nc.vector.tensor_scalar_add(xT[:], xT[:], w["fc_b"][:EMB, :])
            xsq = work.tile([EMB, NPC], f32, tag="xsq", bufs=1)
            nc.vector.tensor_mul(xsq[:], xT[:], xT[:])
            ones8 = wp.tile([EMB, 1], f32)
            nc.gpsimd.memset(ones8[:], 1.0)
            sigb = wp.tile([128, 1], f32)
            nc.gpsimd.memset(sigb[:], B_DEC)
            pssq = psB.tile([128, 512], f32, tag="m")
            nc.tensor.matmul(pssq[:1, :], ones8[:], xsq[:], start=True, stop=True)
            sq_sb = work.tile([1, NPC], f32, tag="sq_sb", bufs=1)
            nc.vector.tensor_copy(sq_sb[:], pssq[:1, :])
            onesr = wp.tile([1, NPC], f32)
            nc.gpsimd.memset(onesr[:], 1.0)
            n2x = work.tile([EMB, NPC], f32, tag="n2x", bufs=1)
            nc.scalar.mul(n2x[:], xT[:], -2.0)
            for dst in (d_qt, d_qin):
                nc.sync.dma_start(dst[0:8, :], xT[:])
                nc.sync.dma_start(dst[8:9, :], onesr[:])
                nc.sync.dma_start(dst[9:10, :], sq_sb[:])
            nc.sync.dma_start(d_pt[0:8, :], n2x[:])
            nc.sync.dma_start(d_pt[8:9, :], sq_sb[:])
            nc.sync.dma_start(d_pt[9:10, :], onesr[:])
            PT = work.tile([10, NPC], f32, tag="PT", bufs=1)
            nc.sync.dma_start(PT[:], d_pt[:])
            nc.gpsimd.collective_compute(
                "AllGather", mybir.AluOpType.bypass, replica_groups=RG,
                ins=[d_qin[:]], outs=[d_qout[:]])
            QTfull = bigp.tile([10, NCORES, NPC], f32)
            nc.sync.dma_start(QTfull[:], d_qout.rearrange("r p f -> p r f"))
            QTv = QTfull[:].rearrange("p r f -> p (r f)")

            for mb in range(RB):
                for nb in range(8):
                    psd = psA.tile([128, 512], f32, tag="p512")
                    nc.tensor.matmul(psd[:], PT[:, mb * 128:(mb + 1) * 128],
                                     QTv[:, nb * 512:(nb + 1) * 512],
                                     start=True, stop=True)
                    sg = work.tile([128, 512], f32, tag="sig", bufs=3)
                    nc.scalar.activation(sg[:], psd[:], Sigm, scale=W_DEC,
                                         bias=sigb[:])
                    nc.sync.dma_start(
                        d_adj[mb * 128:(mb + 1) * 128, nb * 512:(nb + 1) * 512],
                        sg[:])
    nc.compile()
    return nc


def kernel(nodes, edges, edge_attr, gcl_params, fc_w, fc_b):
    from concourse.bass_utils import run_bass_kernel_spmd

    nodes = np.asarray(nodes, dtype=np.float32)
    tmpl, data = _host_prep(nodes, np.asarray(edges), edge_attr)
    ws, zflags = _prep_weights(gcl_params, fc_w, fc_b)

    key = (tmpl["E_pad"], tuple(sorted(zflags.items())),
           tuple(s for _, s, _ in tmpl["chunks"]))
    if key not in _CACHE:
        _CACHE[key] = _build(tmpl, zflags)
    nc = _CACHE[key]

    in_maps = []
    for c in range(NCORES):
        m = dict(data[c])
        m.update(ws)
        in_maps.append({k: np.ascontiguousarray(v) for k, v in m.items()})
    r = run_bass_kernel_spmd(nc, in_maps, list(range(NCORES)))

    adj = np.concatenate([r.results[c]["adj_rows"] for c in range(NCORES)], axis=0)
    np.fill_diagonal(adj, 0.0)
    x = np.concatenate([r.results[c]["qt_shard"][0:8].T for c in range(NCORES)],
                       axis=0)
    return adj, x


# revision 18
# speedup vs baseline: 1.0614x; 1.0614x over previous
"""Trainium2 Bass kernel for nn_BasicAE (GNN message passing + pairwise decode).

8-core SPMD. Nodes sharded (512/core); edges assigned to the destination-row
core, host-sorted by (row_block, col_block) into a cross-core-uniform padded
template so one program serves all cores. All gathers / segment_sum run on the
TensorEngine via host-built block one-hot matmuls:
  e_hid(ch-major) = A_blk @ RowOneHot + B_cblk @ ColOneHot + w_attr @ attr_row
  m(edge-major)   = relu1_chunk.T @ ew2            (layout flip via lhsT)
  agg.T(ch-major) = m_chunk.T @ AggOneHot          (PSUM-accumulated per block)
B = h@W1b node-major tables are AllGathered (bf16) once per layer.
Decode: d2 = P @ Q.T with K=10 augmentation (P=[-2x,sq,1], Q=[x,1,sq]),
sigmoid fused on ScalarE; diagonal zeroed on host after readback.
"""
import numpy as np
import ml_dtypes

N_NODES = 4096
HID = 128
EMB = 8
NCORES = 8
NPC = N_NODES // NCORES          # 512 nodes per core
RB = NPC // 128                  # 4 row blocks per core
CB = N_NODES // 128              # 32 col blocks (global)
W_DEC = 10.0
B_DEC = -1.0

bf16 = ml_dtypes.bfloat16
_CACHE = {}


def _host_prep(nodes, edges, edge_attr):
    row = np.asarray(edges[0], dtype=np.int64)
    col = np.asarray(edges[1], dtype=np.int64)
    attr = np.asarray(edge_attr, dtype=np.float32).reshape(-1)
    nodes = np.asarray(nodes, dtype=np.float32).reshape(-1)

    core_of = row // NPC
    per_core = []
    counts = np.zeros((NCORES, RB, CB), dtype=np.int64)
    for c in range(NCORES):
        m = core_of == c
        r, cl, at = row[m] - c * NPC, col[m], attr[m]
        rb, cb = r // 128, cl // 128
        order = np.lexsort((cb, rb))
        r, cl, at, rb, cb = r[order], cl[order], at[order], rb[order], cb[order]
        np.add.at(counts[c], (rb, cb), 1)
        per_core.append((r, cl, at, rb, cb))

    G = counts.max(axis=0)
    G = ((G + 3) // 4) * 4                            # [RB, CB]
    blk_sz = ((G.sum(axis=1) + 127) // 128) * 128     # [RB]
    blk_start = np.concatenate([[0], np.cumsum(blk_sz)]).astype(np.int64)
    E_pad = int(blk_start[-1])
    grp_start = np.zeros((RB, CB), dtype=np.int64)
    for b in range(RB):
        grp_start[b] = blk_start[b] + np.concatenate([[0], np.cumsum(G[b])[:-1]])

    # chunk template: per row block, 512-chunks plus one remainder chunk
    chunks = []          # (start, size, rb)
    for b in range(RB):
        s, e = int(blk_start[b]), int(blk_start[b + 1])
        while s < e:
            sz = min(512, e - s)
            chunks.append((s, sz, b))
            s += sz
    col_tmpl = [[] for _ in chunks]
    for ci, (cs, csz, crb) in enumerate(chunks):
        for k in range(CB):
            s, e = int(grp_start[crb, k]), int(grp_start[crb, k] + G[crb, k])
            lo, hi = max(s, cs), min(e, cs + csz)
            if hi > lo:
                col_tmpl[ci].append((k, lo - cs, hi - cs))

    data = []
    for c in range(NCORES):
        r, cl, at, rb, cb = per_core[c]
        fill = grp_start.copy()
        pos = np.zeros(len(r), dtype=np.int64)
        for i in range(len(r)):
            pos[i] = fill[rb[i], cb[i]]
            fill[rb[i], cb[i]] += 1
        row_l = np.zeros(E_pad, dtype=np.int64)
        col_g = np.zeros(E_pad, dtype=np.int64)
        attr_v = np.zeros(E_pad, dtype=np.float32)
        row_full = np.zeros(E_pad, dtype=np.int64)
        real = np.zeros(E_pad, dtype=bool)
        row_l[pos] = r % 128
        col_g[pos] = cl
        attr_v[pos] = at
        row_full[pos] = r + c * NPC
        real[pos] = True
        idx = np.nonzero(real)[0]

        row_oh = np.zeros((128, E_pad), dtype=bf16)
        col_oh = np.zeros((128, E_pad), dtype=bf16)
        agg_oh = np.zeros((128, E_pad), dtype=bf16)
        row_oh[row_l[idx], idx] = 1
        col_oh[col_g[idx] % 128, idx] = 1
        agg_oh[idx % 128, (idx // 128) * 128 + row_l[idx]] = 1

        e0 = np.zeros((4, E_pad), dtype=bf16)
        e0[0] = attr_v.astype(bf16)
        e0[1, idx] = nodes[row_full[idx]].astype(bf16)
        e0[2, idx] = nodes[col_g[idx]].astype(bf16)
        data.append(dict(row_oh=row_oh, col_oh=col_oh, agg_oh=agg_oh, e0=e0,
                         nodesT=nodes[c * NPC:(c + 1) * NPC][None, :].astype(bf16)))
    tmpl = dict(E_pad=E_pad, chunks=chunks, col_tmpl=col_tmpl)
    return tmpl, data


def _prep_weights(gcl_params, fc_w, fc_b):
    ws, zflags = {}, {}
    for l, p in enumerate(gcl_params):
        g = {k: np.asarray(v, dtype=np.float32) for k, v in p.items()}
        if l == 0:
            e1 = np.zeros((4, HID), np.float32)
            e1[0] = g["ew1"][2]
            e1[1] = g["ew1"][0]
            e1[2] = g["ew1"][1]
            ws[f"ew1_{l}"] = e1.astype(bf16)
            ws[f"nw1h_{l}"] = g["nw1"][0:1].astype(bf16)
            ws[f"nw1a_{l}"] = g["nw1"][1:129].astype(bf16)
        else:
            ws[f"w1a_{l}"] = g["ew1"][0:HID].astype(bf16)
            ws[f"w1b_{l}"] = g["ew1"][HID:2 * HID].astype(bf16)
            ws[f"wat_{l}"] = g["ew1"][2 * HID:2 * HID + 1].astype(bf16)
            ws[f"nw1h_{l}"] = g["nw1"][0:HID].astype(bf16)
            ws[f"nw1a_{l}"] = g["nw1"][HID:2 * HID].astype(bf16)
        ws[f"ew2_{l}"] = g["ew2"].astype(bf16)
        ws[f"nw2_{l}"] = g["nw2"].astype(bf16)
        ws[f"eb1_{l}"] = np.ascontiguousarray(g["eb1"][:, None])
        ws[f"nb1_{l}"] = np.ascontiguousarray(g["nb1"][:, None])
        ws[f"nb2_{l}"] = np.ascontiguousarray(g["nb2"][:, None])
        ws[f"eb2row_{l}"] = np.ascontiguousarray(g["eb2"][None, :]).astype(bf16)
        zflags[f"eb2_{l}"] = bool(np.all(g["eb2"] == 0))
        zflags[f"nb2_{l}"] = bool(np.all(g["nb2"] == 0))
    ws["fc_w"] = np.asarray(fc_w, np.float32).astype(bf16)
    ws["fc_b"] = np.ascontiguousarray(np.asarray(fc_b, np.float32)[:, None])
    zflags["fc_b"] = bool(np.all(ws["fc_b"] == 0))
    return ws, zflags


def _wspec():
    import concourse.mybir as mybir
    f32, b16 = mybir.dt.float32, mybir.dt.bfloat16
    sp = []
    for l in range(4):
        if l == 0:
            sp += [(f"ew1_{l}", [4, HID], b16), (f"nw1h_{l}", [1, HID], b16)]
        else:
            sp += [(f"w1a_{l}", [HID, HID], b16), (f"w1b_{l}", [HID, HID], b16),
                   (f"wat_{l}", [1, HID], b16), (f"nw1h_{l}", [HID, HID], b16)]
        sp += [(f"nw1a_{l}", [HID, HID], b16), (f"ew2_{l}", [HID, HID], b16),
               (f"nw2_{l}", [HID, HID], b16), (f"eb1_{l}", [HID, 1], f32),
               (f"nb1_{l}", [HID, 1], f32), (f"nb2_{l}", [HID, 1], f32),
               (f"eb2row_{l}", [1, HID], b16)]
    sp += [("fc_w", [HID, EMB], b16), ("fc_b", [EMB, 1], f32)]
    return sp


def _build(tmpl, zflags):
    import concourse.mybir as mybir
    import concourse.tile as tile
    from concourse import bacc
    from concourse.masks import make_identity

    E_pad = tmpl["E_pad"]
    chunks = tmpl["chunks"]
    col_tmpl = tmpl["col_tmpl"]
    f32, b16 = mybir.dt.float32, mybir.dt.bfloat16
    Relu = mybir.ActivationFunctionType.Relu
    Sigm = mybir.ActivationFunctionType.Sigmoid
    Copy = mybir.ActivationFunctionType.Copy

    nc = bacc.Bacc(None, target_bir_lowering=False, debug=False)

    d_rowoh = nc.dram_tensor("row_oh", [128, E_pad], b16, kind="ExternalInput")
    d_coloh = nc.dram_tensor("col_oh", [128, E_pad], b16, kind="ExternalInput")
    d_aggoh = nc.dram_tensor("agg_oh", [128, E_pad], b16, kind="ExternalInput")
    d_e0 = nc.dram_tensor("e0", [4, E_pad], b16, kind="ExternalInput")
    d_nodesT = nc.dram_tensor("nodesT", [1, NPC], b16, kind="ExternalInput")
    wspec = _wspec()
    d_w = {nm: nc.dram_tensor(nm, shp, dt, kind="ExternalInput")
           for nm, shp, dt in wspec}
    d_adj = nc.dram_tensor("adj_rows", [NPC, N_NODES], f32, kind="ExternalOutput")
    d_qt = nc.dram_tensor("qt_shard", [10, NPC], f32, kind="ExternalOutput")

    d_warm_in = nc.dram_tensor("warm_in", [1, 64], f32)
    d_warm_out = nc.dram_tensor("warm_out", [NCORES, 64], f32, addr_space="Shared")
    d_bsh = {l: nc.dram_tensor(f"bsh_{l}", [NPC, HID], b16) for l in (1, 2, 3)}
    d_btab = {l: nc.dram_tensor(f"btab_{l}", [NCORES, NPC, HID], b16,
                                addr_space="Shared") for l in (1, 2, 3)}
    d_qin = nc.dram_tensor("q_in", [10, NPC], f32)
    d_pt = nc.dram_tensor("p_t", [10, NPC], f32)
    d_qout = nc.dram_tensor("q_out", [NCORES, 10, NPC], f32, addr_space="Shared")
    RG = [list(range(NCORES))]

    with tile.TileContext(nc) as tc:
        with (
            tc.tile_pool(name="big", bufs=1) as bigp,
            tc.tile_pool(name="wp", bufs=1) as wp,
            tc.tile_pool(name="work", bufs=2) as work,
            tc.tile_pool(name="psA", bufs=3, space="PSUM") as psA,
            tc.tile_pool(name="psB", bufs=2, space="PSUM") as psB,
            tc.tile_pool(name="psT", bufs=1, space="PSUM") as psT,
            tc.tile_pool(name="psG", bufs=1, space="PSUM") as psG,
        ):
            zsm = wp.tile([1, 64], f32)
            nc.gpsimd.memset(zsm[:], 0.0)
            nc.sync.dma_start(d_warm_in[:], zsm[:])
            nc.gpsimd.collective_compute(
                "AllGather", mybir.AluOpType.bypass, replica_groups=RG,
                ins=[d_warm_in[:]], outs=[d_warm_out[:]])
            row_oh = bigp.tile([128, E_pad], b16)
            col_oh = bigp.tile([128, E_pad], b16)
            e0 = bigp.tile([4, E_pad], b16)
            tableB = bigp.tile([128, CB, 128], b16)
            nc.sync.dma_start(e0[:], d_e0[:])
            nodesT = wp.tile([1, NPC], b16)
            nc.sync.dma_start(nodesT[:], d_nodesT[:])
            w = {}
            for nm, shp, dt in wspec:
                w[nm] = wp.tile(shp, dt, tag=nm, name=nm)
                nc.sync.dma_start(w[nm][:], d_w[nm][:])
            ident = wp.tile([128, 128], b16)
            make_identity(nc, ident)
            ones1 = wp.tile([1, 128], b16)
            nc.gpsimd.memset(ones1[:], 1.0)
            hT = nodesT
            for l in range(4):
                if l == 1:
                    nc.sync.dma_start(row_oh[:], d_rowoh[:])
                    nc.sync.dma_start(col_oh[:], d_coloh[:])
                if l > 0:
                    psa = psA.tile([128, 512], f32, tag="p512")
                    nc.tensor.matmul(psa[:], w[f"w1a_{l}"][:], hT[:],
                                     start=True, stop=True)
                    aT = work.tile([128, NPC], b16, tag="aT")
                    nc.scalar.activation(aT[:], psa[:], Copy)
                    tableA = work.tile([128, RB, 128], b16, tag="tableA")
                    for k in range(RB):
                        pst = psT.tile([128, 128], b16, tag="tp")
                        nc.tensor.matmul(pst[:], aT[:, k * 128:(k + 1) * 128],
                                         ident[:], is_transpose=True,
                                         start=True, stop=True)
                        nc.vector.tensor_copy(tableA[:, k, :], pst[:])

                agg_ps = psG.tile([128, RB * 128], f32, tag="agg")
                aggoh_blk = None
                blk0 = 0
                for ci, (cs, csz, crb) in enumerate(chunks):
                    sl = slice(cs, cs + csz)
                    if ci == 0 or chunks[ci - 1][2] != crb:
                        blk0 = cs
                        bsz = sum(z for (_, z, b) in chunks if b == crb)
                        mxb = max(sum(z for (_, z, b) in chunks if b == bb)
                                  for bb in range(RB))
                        aggoh_blk = work.tile([128, mxb], b16, tag="aggohB",
                                              bufs=2)
                        nc.sync.dma_start(aggoh_blk[:, :bsz],
                                          d_aggoh[:, blk0:blk0 + bsz])
                    ps = psA.tile([128, 512], f32, tag="p512")
                    if l == 0:
                        nc.tensor.matmul(ps[:, :csz], w["ew1_0"][:], e0[:, sl],
                                         start=True, stop=True)
                    else:
                        nc.tensor.matmul(ps[:, :csz], tableA[:, crb, :],
                                         row_oh[:, sl], start=True, stop=False)
                        nc.tensor.matmul(ps[:, :csz], w[f"wat_{l}"][:],
                                         e0[0:1, sl], start=False, stop=False)
                        segs = col_tmpl[ci]
                        for i, (cb, lo, hi) in enumerate(segs):
                            nc.tensor.matmul(
                                ps[:, lo:hi], tableB[:, cb, :],
                                col_oh[:, cs + lo:cs + hi],
                                start=False, stop=(i == len(segs) - 1))
                    relu1 = work.tile([128, 512], b16, tag="relu1", bufs=3)
                    nc.scalar.activation(relu1[:, :csz], ps[:, :csz], Relu,
                                         bias=w[f"eb1_{l}"][:])
                    aggoh_t = aggoh_blk[:, cs - blk0:cs - blk0 + csz]
                    ps2 = psB.tile([128, 512], f32, tag="m")
                    nsub = csz // 128
                    for k in range(nsub):
                        ksl = slice(k * 128, (k + 1) * 128)
                        nc.tensor.matmul(ps2[:, ksl], relu1[:, ksl],
                                         w[f"ew2_{l}"][:], start=True,
                                         stop=zflags[f"eb2_{l}"])
                        if not zflags[f"eb2_{l}"]:
                            nc.tensor.matmul(ps2[:, ksl], ones1[:],
                                             w[f"eb2row_{l}"][:],
                                             start=False, stop=True)
                    mt = work.tile([128, 512], b16, tag="m_sb", bufs=3)
                    nc.scalar.activation(mt[:, :csz], ps2[:, :csz], Relu)
                    first = ci == 0 or chunks[ci - 1][2] != crb
                    last = ci == len(chunks) - 1 or chunks[ci + 1][2] != crb
                    for k in range(nsub):
                        ksl = slice(k * 128, (k + 1) * 128)
                        nc.tensor.matmul(agg_ps[:, crb * 128:(crb + 1) * 128],
                                         mt[:, ksl], aggoh_t[:, ksl],
                                         start=(first and k == 0),
                                         stop=(last and k == nsub - 1))
                aggT = work.tile([128, NPC], b16, tag="aggT")
                nc.scalar.activation(aggT[:], agg_ps[:], Copy)

                psh = psA.tile([128, 512], f32, tag="p512")
                nc.tensor.matmul(psh[:], w[f"nw1h_{l}"][:], hT[:],
                                 start=True, stop=False)
                nc.tensor.matmul(psh[:], w[f"nw1a_{l}"][:], aggT[:],
                                 start=False, stop=True)
                hid = work.tile([128, NPC], b16, tag="nhid")
                nc.scalar.activation(hid[:], psh[:], Relu, bias=w[f"nb1_{l}"][:])
                psh2 = psB.tile([128, 512], f32, tag="m")
                nc.tensor.matmul(psh2[:], w[f"nw2_{l}"][:], hid[:],
                                 start=True, stop=True)
                hT_new = work.tile([128, NPC], b16, tag=f"hT_{l % 2}")
                nc.scalar.activation(hT_new[:], psh2[:], Copy)
                if not zflags[f"nb2_{l}"]:
                    nc.vector.tensor_scalar_add(hT_new[:], hT_new[:],
                                                w[f"nb2_{l}"][:])
                hT = hT_new

                if l < 3:
                    lb = l + 1
                    psb = psA.tile([128, 512], f32, tag="p512")
                    nc.tensor.matmul(psb[:], w[f"w1b_{lb}"][:], hT[:],
                                     start=True, stop=True)
                    bT = work.tile([128, NPC], b16, tag="bT")
                    nc.scalar.activation(bT[:], psb[:], Copy)
                    bnm = work.tile([128, RB, 128], b16, tag="bnm")
                    for k in range(RB):
                        pst = psT.tile([128, 128], b16, tag="tp")
                        nc.tensor.matmul(pst[:], bT[:, k * 128:(k + 1) * 128],
                                         ident[:], is_transpose=True,
                                         start=True, stop=True)
                        nc.vector.tensor_copy(bnm[:, k, :], pst[:])
                    nc.sync.dma_start(
                        d_bsh[lb].rearrange("(k p) c -> p k c", p=128), bnm[:])
                    nc.gpsimd.collective_compute(
                        "AllGather", mybir.AluOpType.bypass, replica_groups=RG,
                        ins=[d_bsh[lb][:]], outs=[d_btab[lb][:]])
                    nc.sync.dma_start(
                        tableB[:].rearrange("p (r k) c -> p r k c", r=NCORES),
                        d_btab[lb].rearrange("r (k p) c -> p r k c", p=128))

            # ---- decode ----
            psx = psA.tile([128, 512], f32, tag="p512")
            nc.tensor.matmul(psx[:EMB, :], w["fc_w"][:], hT[:],
                             start=True, stop=True)
            xT = work.tile([EMB, NPC], f32, tag="xT", bufs=1)
            nc.scalar.activation(xT[:], psx[:EMB, :], Copy)
            if not zflags["fc_b"]:
                nc.vector.tensor_scalar_add(xT[:], xT[:], w["fc_b"][:EMB, :])
            xsq = work.tile([EMB, NPC], f32, tag="xsq", bufs=1)
            nc.vector.tensor_mul(xsq[:], xT[:], xT[:])
            ones8 = wp.tile([EMB, 1], f32)
            nc.gpsimd.memset(ones8[:], 1.0)
            sigb = wp.tile([128, 1], f32)
            nc.gpsimd.memset(sigb[:], B_DEC)
            pssq = psB.tile([128, 512], f32, tag="m")
            nc.tensor.matmul(pssq[:1, :], ones8[:], xsq[:], start=True, stop=True)
            sq_sb = work.tile([1, NPC], f32, tag="sq_sb", bufs=1)
            nc.vector.tensor_copy(sq_sb[:], pssq[:1, :])
            onesr = wp.tile([1, NPC], f32)
            nc.gpsimd.memset(onesr[:], 1.0)
            n2x = work.tile([EMB, NPC], f32, tag="n2x", bufs=1)
            nc.scalar.mul(n2x[:], xT[:], -2.0)
            for dst in (d_qt, d_qin):
                nc.sync.dma_start(dst[0:8, :], xT[:])
                nc.sync.dma_start(dst[8:9, :], onesr[:])
                nc.sync.dma_start(dst[9:10, :], sq_sb[:])
            nc.sync.dma_start(d_pt[0:8, :], n2x[:])
            nc.sync.dma_start(d_pt[8:9, :], sq_sb[:])
            nc.sync.dma_start(d_pt[9:10, :], onesr[:])
            PT = work.tile([10, NPC], f32, tag="PT", bufs=1)
            nc.sync.dma_start(PT[:], d_pt[:])
            nc.gpsimd.collective_compute(
                "AllGather", mybir.AluOpType.bypass, replica_groups=RG,
                ins=[d_qin[:]], outs=[d_qout[:]])
            QTfull = bigp.tile([10, NCORES, NPC], f32)
            nc.sync.dma_start(QTfull[:], d_qout.rearrange("r p f -> p r f"))
            QTv = QTfull[:].rearrange("p r f -> p (r f)")

            for mb in range(RB):
                for nb in range(8):
                    psd = psA.tile([128, 512], f32, tag="p512")
                    nc.tensor.matmul(psd[:], PT[:, mb * 128:(mb + 1) * 128],
                                     QTv[:, nb * 512:(nb + 1) * 512],
                                     start=True, stop=True)
                    sg = work.tile([128, 512], f32, tag="sig", bufs=3)
                    nc.scalar.activation(sg[:], psd[:], Sigm, scale=W_DEC,
                                         bias=sigb[:])
                    nc.sync.dma_start(
                        d_adj[mb * 128:(mb + 1) * 128, nb * 512:(nb + 1) * 512],
                        sg[:])
    nc.compile()
    return nc


def kernel(nodes, edges, edge_attr, gcl_params, fc_w, fc_b):
    from concourse.bass_utils import run_bass_kernel_spmd

    nodes = np.asarray(nodes, dtype=np.float32)
    tmpl, data = _host_prep(nodes, np.asarray(edges), edge_attr)
    ws, zflags = _prep_weights(gcl_params, fc_w, fc_b)

    key = (tmpl["E_pad"], tuple(sorted(zflags.items())),
           tuple(s for _, s, _ in tmpl["chunks"]))
    if key not in _CACHE:
        _CACHE[key] = _build(tmpl, zflags)
    nc = _CACHE[key]

    in_maps = []
    for c in range(NCORES):
        m = dict(data[c])
        m.update(ws)
        in_maps.append({k: np.ascontiguousarray(v) for k, v in m.items()})
    r = run_bass_kernel_spmd(nc, in_maps, list(range(NCORES)))

    adj = np.concatenate([r.results[c]["adj_rows"] for c in range(NCORES)], axis=0)
    np.fill_diagonal(adj, 0.0)
    x = np.concatenate([r.results[c]["qt_shard"][0:8].T for c in range(NCORES)],
                       axis=0)
    return adj, x
